# revision 14
# baseline (speedup 1.0000x reference)
"""Multi-head attention (b=4, n=2048, dm=1024, h=16) on 8 TRN2 NeuronCores.

Sharding: batch (4) x head-group (2) -> 8 cores, Megatron-style.
Core c handles batch c//2 and heads [8*(c%2), 8*(c%2)+8).

v3 design (vs v1 baseline at 616us):
  * Stage 1 QKV projection in f32r (same PE speed as bf16 at N=512) fed
    directly from DMA'd f32 x and weights -> no input casts at all.
    psum->bf16 output casts split between DVE and the scalar engine
    (scalar is otherwise idle in stage 1).
  * Stage 2 attention processes queries in 512-wide quarters so every
    PSUM tile is 1 bank: scores (2 heads) double-buffered = 4 banks,
    ctx accumulators double-buffered = 4 banks.  The PE never waits for
    softmax to drain a psum bank, keeping it continuously busy -- on
    TRN2 the PE p-state ramps to 2.4 GHz only after ~3us of gap-free
    execution and resets on every idle gap (this, not the "activity
    monitor", is why v1 ran at ~1.2 GHz: its single-buffered score psum
    stalled the PE on every key-chunk).  No heater matmuls.
  * Score matmuls issue one key-chunk ahead of ctx matmuls (software
    pipeline) so the PE instruction queue never head-of-line blocks on
    a not-yet-exp'd tile.
  * exp(S*scale) split across two engines: scalar (native Exp, 2/3)
    and DVE (1/3) via a bf16 Schraudolph bit trick
    (int16 = S*A16 + B16 is the bf16 bit pattern of exp(S*SCALE),
    ~1.8% rms after softmax bias cancellation).  33.5M exp elements at
    153G/s on scalar alone would be a 220us+ bottleneck.
  * ctx^T = v'^T E per head with a ones column producing the softmax
    denominator for free (M=65); denominator broadcast by a K=1 ones
    matmul; reciprocal_approx_fast + multiply on DVE (as v1).
  * Stage 3 output projection in bf16 (w_out pre-cast to bf16 bits on
    the host); psum->f32 copies on the scalar engine (idle by then).
Host sums the two partials per batch and adds the bias.
"""

import numpy as np

import concourse.bass as bass
import concourse.tile as tile
from concourse import bacc, mybir
from concourse import bass_utils

f32 = mybir.dt.float32
f32r = mybir.dt.float32r
f8 = mybir.dt.float8e4
bf16 = mybir.dt.bfloat16
i16 = mybir.dt.int16
u16 = mybir.dt.uint16
Exp = mybir.ActivationFunctionType.Exp
Copy = mybir.ActivationFunctionType.Copy
Mult = mybir.AluOpType.mult
Add = mybir.AluOpType.add

TOK = 2048          # tokens per batch
DM = 1024           # model dim
DL = 512            # local q/k/v feature dim (8 heads x 64)
D = 64              # head dim
NH = 8              # local heads
NPAIR = 4           # head pairs (partition blocks of ctx/qk)
KT = 8              # dm / 128 contraction tiles
SCALE = DM ** (-0.5)
N_CORES = 8

# bf16 Schraudolph: bf16bits(exp(s*SCALE)) ~= trunc(s * A16 + B16).
# The -6.75 zeroes the trick's mean multiplicative bias (so scalar-exp'd
# and trick'd key-chunks weight consistently inside one softmax row);
# it splits the difference between truncating (-6.5) and rounding (-7.0)
# f32->i16 conversion.
A16 = 128.0 * SCALE / float(np.log(2.0))
B16 = 127.0 * 128.0 - 6.75

# exp engine schedule per tile: all-scalar minimizes total engine-seconds
# (the power governor clamps the PE based on total cross-engine activity,
# and the DVE bit-trick costs ~28% more engine-time per element than the
# scalar engine's native exp).  "d" entries would route tiles to the DVE
# Schraudolph path.  (gpsimd cannot access PSUM, so it cannot help.)
EXP_SCHED = ("s",)


def _build(tc, xT, wqkT, wvT, wo16, out_p):
    nc = tc.nc

    ctp = tc.alloc_tile_pool(name="ctp", bufs=1)
    qkp = tc.alloc_tile_pool(name="qkp", bufs=1)
    vp_ = tc.alloc_tile_pool(name="vp", bufs=1)

    ctx_sb = ctp.tile([128, NPAIR, TOK], bf16, tag="ctx")    # 16 KB/part
    qk_sb = qkp.tile([128, 2 * NPAIR, TOK], bf16, tag="qk")  # 32 KB/part
    v_sb = vp_.tile([128, 16, NH, D + 1], bf16, tag="v")     # 17 KB/part

    # ones column of v' (softmax denominator accumulator)
    nc.vector.memset(v_sb[:, :, :, D:D + 1], 1.0)

    # ---- stage 1: QKV projection in f32r, token-quarters of 512 ----
    with (
        tc.tile_pool(name="w1", bufs=1) as wp,
        tc.tile_pool(name="x", bufs=2) as xp,
        tc.tile_pool(name="psq", bufs=4, space="PSUM") as psq,
    ):
        wqk_sb = wp.tile([128, KT, 2 * DL], f32r, tag="wqk")  # 32 KB/part
        wv_sb = wp.tile([128, KT, DL], f32r, tag="wv")        # 16 KB/part
        for kt in range(KT):
            nc.sync.dma_start(wqk_sb[:, kt, :], wqkT[:, kt, :])
            nc.sync.dma_start(wv_sb[:, kt, :], wvT[:, kt, :])
        ncast = 0
        for tq in range(4):
            ts512 = slice(tq * 512, (tq + 1) * 512)
            xt = xp.tile([128, KT, 512], f32r, tag="x")
            for kt in range(KT):
                nc.sync.dma_start(xt[:, kt, :], xT[:, kt, ts512])
            # q/k (transposed layout): out [feat 128, tok 512]
            for f in range(8):
                ps = psq.tile([128, 512], f32, tag="qk")
                for kt in range(KT):
                    nc.tensor.matmul(
                        ps[:], wqk_sb[:, kt, f * 128:(f + 1) * 128],
                        xt[:, kt, :],
                        start=(kt == 0), stop=(kt == KT - 1))
                with nc.allow_low_precision(reason="bf16"):
                    if ncast % 2 == 0:
                        nc.vector.tensor_copy(qk_sb[:, f, ts512], ps[:])
                    else:
                        nc.scalar.activation(qk_sb[:, f, ts512], ps[:], Copy)
                ncast += 1
            # v (natural layout): out [tok 128, feat 512]
            for tt in range(4):
                jt = tq * 4 + tt
                ps = psq.tile([128, 512], f32, tag="v")
                for kt in range(KT):
                    nc.tensor.matmul(
                        ps[:], xt[:, kt, tt * 128:(tt + 1) * 128],
                        wv_sb[:, kt, :],
                        start=(kt == 0), stop=(kt == KT - 1))
                with nc.allow_low_precision(reason="bf16 v"):
                    nc.vector.tensor_copy(
                        v_sb[:, jt, :, 0:D],
                        ps[:].rearrange("p (h d) -> p h d", h=NH))

    # ---- stage 2: attention, per head-pair, query-halves of 1024 ----
    with (
        tc.tile_pool(name="psS", bufs=1, space="PSUM") as psS,
        tc.tile_pool(name="psC", bufs=1, space="PSUM") as psC,
        tc.tile_pool(name="ep", bufs=2) as ep,
        tc.tile_pool(name="dv", bufs=2) as dv,
        tc.tile_pool(name="on", bufs=1) as onp,
    ):
        # ones row at partition 64 for the K=1 denominator-broadcast matmul
        ones_t = onp.tile([65, D], f32r, tag="ones")
        nc.vector.memset(ones_t[:].bitcast(f32), 1.0)
        eidx = 0
        for p in range(4):
            for ih in range(2):
                ihs = slice(ih * 1024, (ih + 1) * 1024)
                ps_ca = psC.tile([65, 1024], f32, tag="CA")
                ps_cb = psC.tile([65, 1024], f32, tag="CB")
                # software pipeline: scores(jt) issue before ctx(jt-1)
                pend = None   # (e_a, e_b) waiting for their ctx matmuls
                for jt in range(16):
                    js = slice(jt * 128, (jt + 1) * 128)
                    ps_sa = psS.tile([128, 1024], f32, tag="SA")
                    ps_sb = psS.tile([128, 1024], f32, tag="SB")
                    for poff, ps_s in ((0, ps_sa), (64, ps_sb)):
                        for ic in range(2):
                            cs = slice(ic * 512, (ic + 1) * 512)
                            qs = slice(ih * 1024 + ic * 512,
                                       ih * 1024 + ic * 512 + 512)
                            nc.tensor.matmul(
                                ps_s[:, cs], qk_sb[poff:poff + D, 4 + p, js],
                                qk_sb[poff:poff + D, p, qs],
                                start=True, stop=True)
                    if pend is not None:
                        for ps_c, e_t, hh in ((ps_ca, pend[0], 0),
                                              (ps_cb, pend[1], 1)):
                            for ic in range(2):
                                cs = slice(ic * 512, (ic + 1) * 512)
                                nc.tensor.matmul(
                                    ps_c[:, cs], v_sb[:, jt - 1, 2 * p + hh, :],
                                    e_t[:, cs], start=(jt == 1),
                                    stop=(jt == 16))
                    e_a = ep.tile([128, 1024], bf16, tag="EA")
                    e_b = ep.tile([128, 1024], bf16, tag="EB")
                    for ps_s, e_t in ((ps_sa, e_a), (ps_sb, e_b)):
                        eng = EXP_SCHED[eidx % len(EXP_SCHED)]
                        eidx += 1
                        with nc.allow_low_precision(reason="exp"):
                            if eng == "s":
                                nc.scalar.activation(e_t[:], ps_s[:], Exp,
                                                     scale=SCALE)
                            else:
                                nc.vector.tensor_scalar(
                                    e_t[:].bitcast(i16), ps_s[:],
                                    A16, B16, Mult, Add)
                    pend = (e_a, e_b)
                # drain last key-chunk's ctx
                for ps_c, e_t, hh in ((ps_ca, pend[0], 0), (ps_cb, pend[1], 1)):
                    for ic in range(2):
                        cs = slice(ic * 512, (ic + 1) * 512)
                        nc.tensor.matmul(
                            ps_c[:, cs], v_sb[:, 15, 2 * p + hh, :],
                            e_t[:, cs], start=False, stop=True)
                # softmax division per head (denominator = row 64 of ctx psum)
                for s, ps_c in ((0, ps_ca), (1, ps_cb)):
                    den = dv.tile([65, 1024], f32r, tag="den", name=f"den{s}")
                    with nc.allow_low_precision(reason="f32r denom"):
                        nc.vector.tensor_copy(den[64:65, :], ps_c[64:65, :])
                    bc = psS.tile([64, 1024], f32, tag="SA", name=f"bc{s}")
                    for ic in range(2):
                        cs = slice(ic * 512, (ic + 1) * 512)
                        nc.tensor.matmul(bc[:, cs], ones_t[64:65, :],
                                         den[64:65, cs], start=True, stop=True)
                    rec = dv.tile([64, 1024], f32, tag="rec", name=f"rec{s}")
                    nc.vector.reciprocal_approx_fast(rec[:], bc[:])
                    with nc.allow_low_precision(reason="bf16 ctx"):
                        if s == 0:
                            nc.vector.tensor_tensor(
                                out=ctx_sb[0:D, p, ihs], in0=ps_c[0:D, :],
                                in1=rec[:], op=Mult)
                        else:
                            tmp = dv.tile([64, 1024], bf16, tag="tmp")
                            nc.vector.tensor_tensor(
                                out=tmp[:], in0=ps_c[0:D, :], in1=rec[:],
                                op=Mult)
                            # shift to partitions 64:128 via SBUF->SBUF DMA
                            nc.sync.dma_start(ctx_sb[64:128, p, ihs], tmp[:])

    vp_.release()
    qkp.release()

    # ---- stage 3: output projection (bf16) ----
    with (
        tc.tile_pool(name="w3", bufs=1) as w3,
        tc.tile_pool(name="psO", bufs=2, space="PSUM") as psO,
        tc.tile_pool(name="ot", bufs=3) as otp,
    ):
        wout_sb = w3.tile([128, NPAIR, DM], u16, tag="wout")  # 8 KB/part
        nc.sync.dma_start(wout_sb[:], wo16[:])
        for tt in range(16):
            o_t = otp.tile([128, DM], f32, tag="o")
            for fc in range(2):
                fs = slice(fc * 512, (fc + 1) * 512)
                ps = psO.tile([128, 512], f32, tag="O")
                for pb in range(NPAIR):
                    nc.tensor.matmul(
                        ps[:], ctx_sb[:, pb, tt * 128:(tt + 1) * 128],
                        wout_sb[:, pb, fs].bitcast(bf16),
                        start=(pb == 0), stop=(pb == NPAIR - 1))
                nc.scalar.activation(o_t[:, fs], ps[:], Copy)
            nc.sync.dma_start(out_p[tt * 128:(tt + 1) * 128, :], o_t[:])
    ctp.release()


_CACHE = {}


def _get_nc():
    if "nc" not in _CACHE:
        nc = bacc.Bacc("TRN2", target_bir_lowering=False, debug=False)
        xT = nc.dram_tensor("xT", [128, KT, TOK], f32r, kind="ExternalInput").ap()
        wqkT = nc.dram_tensor("wqkT", [128, KT, 2 * DL], f32r,
                              kind="ExternalInput").ap()
        wvT = nc.dram_tensor("wvT", [128, KT, DL], f32r,
                             kind="ExternalInput").ap()
        wo16 = nc.dram_tensor("wo16", [128, NPAIR, DM], u16,
                              kind="ExternalInput").ap()
        out_p = nc.dram_tensor("out_p", [TOK, DM], f32, kind="ExternalOutput").ap()
        with tile.TileContext(nc) as tc:
            _build(tc, xT, wqkT, wvT, wo16, out_p)
        nc.compile()
        _CACHE["nc"] = nc
    return _CACHE["nc"]


def _bf16_bits(x):
    b = np.ascontiguousarray(np.asarray(x, np.float32)).view(np.uint32)
    return ((b + 0x7FFF + ((b >> 16) & 1)) >> 16).astype(np.uint16)


def _fold(a):
    """[DM, cols] -> [128, KT, cols] with partition-major dm chunks."""
    return np.ascontiguousarray(
        a.reshape(KT, 128, a.shape[1]).transpose(1, 0, 2))


def make_in_maps(x, w_qkv, w_out):
    in_maps = []
    xTb = {b: _fold(np.ascontiguousarray(x[b].T)) for b in range(4)}
    for c in range(N_CORES):
        b, g = c // 2, c % 2
        gs = slice(g * DL, (g + 1) * DL)
        wq = w_qkv[0 * DM + g * DL:0 * DM + (g + 1) * DL]
        wk = w_qkv[1 * DM + g * DL:1 * DM + (g + 1) * DL]
        wv = w_qkv[2 * DM + g * DL:2 * DM + (g + 1) * DL]
        woT = np.ascontiguousarray(w_out[:, gs].T)        # [DL, DM]
        wo16 = np.ascontiguousarray(
            _bf16_bits(woT).reshape(NPAIR, 128, DM).transpose(1, 0, 2))
        in_maps.append({
            "xT": xTb[b],
            "wqkT": _fold(np.ascontiguousarray(np.concatenate([wq, wk], 0).T)),
            "wvT": _fold(np.ascontiguousarray(wv.T)),
            "wo16": wo16,
        })
    return in_maps


def kernel(x, w_qkv, w_out, b_out, _trace=False):
    x = np.asarray(x, dtype=np.float32)
    w_qkv = np.asarray(w_qkv, dtype=np.float32)
    w_out = np.asarray(w_out, dtype=np.float32)
    b_out = np.asarray(b_out, dtype=np.float32)

    nc = _get_nc()
    in_maps = make_in_maps(x, w_qkv, w_out)
    res = bass_utils.run_bass_kernel_spmd(
        nc, in_maps, core_ids=list(range(N_CORES)), trace=_trace)
    out = np.empty((4, TOK, DM), dtype=np.float32)
    for b in range(4):
        out[b] = res.results[2 * b]["out_p"] + res.results[2 * b + 1]["out_p"]
    out += b_out
    if _trace:
        kernel.last_results = res
    return out


# revision 15
# speedup vs baseline: 1.1173x; 1.1173x over previous
"""Multi-head attention (b=4, n=2048, dm=1024, h=16) on 8 TRN2 NeuronCores.

Sharding: batch (4) x head-group (2) -> 8 cores, Megatron-style.
Core c handles batch c//2 and heads [8*(c%2), 8*(c%2)+8).

v3 design (vs v1 baseline at 616us):
  * Stage 1 QKV projection in f32r (same PE speed as bf16 at N=512) fed
    directly from DMA'd f32 x and weights -> no input casts at all.
    psum->bf16 output casts split between DVE and the scalar engine
    (scalar is otherwise idle in stage 1).
  * Stage 2 attention processes queries in 512-wide quarters so every
    PSUM tile is 1 bank: scores (2 heads) double-buffered = 4 banks,
    ctx accumulators double-buffered = 4 banks.  The PE never waits for
    softmax to drain a psum bank, keeping it continuously busy -- on
    TRN2 the PE p-state ramps to 2.4 GHz only after ~3us of gap-free
    execution and resets on every idle gap (this, not the "activity
    monitor", is why v1 ran at ~1.2 GHz: its single-buffered score psum
    stalled the PE on every key-chunk).  No heater matmuls.
  * Score matmuls issue one key-chunk ahead of ctx matmuls (software
    pipeline) so the PE instruction queue never head-of-line blocks on
    a not-yet-exp'd tile.
  * exp(S*scale) split across two engines: scalar (native Exp, 2/3)
    and DVE (1/3) via a bf16 Schraudolph bit trick
    (int16 = S*A16 + B16 is the bf16 bit pattern of exp(S*SCALE),
    ~1.8% rms after softmax bias cancellation).  33.5M exp elements at
    153G/s on scalar alone would be a 220us+ bottleneck.
  * ctx^T = v'^T E per head with a ones column producing the softmax
    denominator for free (M=65); denominator broadcast by a K=1 ones
    matmul; reciprocal_approx_fast + multiply on DVE (as v1).
  * Stage 3 output projection in bf16 (w_out pre-cast to bf16 bits on
    the host); psum->f32 copies on the scalar engine (idle by then).
Host sums the two partials per batch and adds the bias.
"""

import numpy as np

import concourse.bass as bass
import concourse.tile as tile
from concourse import bacc, mybir
from concourse import bass_utils

f32 = mybir.dt.float32
f32r = mybir.dt.float32r
f8 = mybir.dt.float8e4
bf16 = mybir.dt.bfloat16
i16 = mybir.dt.int16
u16 = mybir.dt.uint16
Exp = mybir.ActivationFunctionType.Exp
Copy = mybir.ActivationFunctionType.Copy
Mult = mybir.AluOpType.mult
Add = mybir.AluOpType.add

TOK = 2048          # tokens per batch
DM = 1024           # model dim
DL = 512            # local q/k/v feature dim (8 heads x 64)
D = 64              # head dim
NH = 8              # local heads
NPAIR = 4           # head pairs (partition blocks of ctx/qk)
KT = 8              # dm / 128 contraction tiles
SCALE = DM ** (-0.5)
N_CORES = 8

# bf16 Schraudolph: bf16bits(exp(s*SCALE)) ~= trunc(s * A16 + B16).
# The -6.75 zeroes the trick's mean multiplicative bias (so scalar-exp'd
# and trick'd key-chunks weight consistently inside one softmax row);
# it splits the difference between truncating (-6.5) and rounding (-7.0)
# f32->i16 conversion.
A16 = 128.0 * SCALE / float(np.log(2.0))
B16 = 127.0 * 128.0 - 6.75

# exp engine schedule, cycled per tile: 4/5 scalar, 1/5 DVE.  The DVE
# trick costs ~28% more engine-time per tile than the scalar's native
# exp, and the power governor clamps on TOTAL cross-engine activity, so
# scalar-heavy minimizes engine-seconds (and Schraudolph error).
# (gpsimd cannot access PSUM on TRN2, so it cannot help with exp.)
EXP_SCHED = ("s", "s", "s", "s", "d")


def _build(tc, xT, wqkT, wvT, wo16, out_p):
    nc = tc.nc

    ctp = tc.alloc_tile_pool(name="ctp", bufs=1)
    qkp = tc.alloc_tile_pool(name="qkp", bufs=1)
    vp_ = tc.alloc_tile_pool(name="vp", bufs=1)

    ctx_sb = ctp.tile([128, NPAIR, TOK], bf16, tag="ctx")    # 16 KB/part
    qk_sb = qkp.tile([128, 2 * NPAIR, TOK], bf16, tag="qk")  # 32 KB/part
    v_sb = vp_.tile([128, 16, NH, D + 1], bf16, tag="v")     # 17 KB/part

    # ones column of v' (softmax denominator accumulator)
    nc.vector.memset(v_sb[:, :, :, D:D + 1], 1.0)

    # ---- stage 1: QKV projection in f32r, token-quarters of 512 ----
    with (
        tc.tile_pool(name="w1", bufs=1) as wp,
        tc.tile_pool(name="x", bufs=2) as xp,
        tc.tile_pool(name="psq", bufs=4, space="PSUM") as psq,
    ):
        wqk_sb = wp.tile([128, KT, 2 * DL], f32r, tag="wqk")  # 32 KB/part
        wv_sb = wp.tile([128, KT, DL], f32r, tag="wv")        # 16 KB/part
        for kt in range(KT):
            nc.sync.dma_start(wqk_sb[:, kt, :], wqkT[:, kt, :])
            nc.sync.dma_start(wv_sb[:, kt, :], wvT[:, kt, :])
        ncast = 0
        for tq in range(4):
            ts512 = slice(tq * 512, (tq + 1) * 512)
            xt = xp.tile([128, KT, 512], f32r, tag="x")
            for kt in range(KT):
                nc.sync.dma_start(xt[:, kt, :], xT[:, kt, ts512])
            # q/k (transposed layout): out [feat 128, tok 512]
            for f in range(8):
                ps = psq.tile([128, 512], f32, tag="qk")
                for kt in range(KT):
                    nc.tensor.matmul(
                        ps[:], wqk_sb[:, kt, f * 128:(f + 1) * 128],
                        xt[:, kt, :],
                        start=(kt == 0), stop=(kt == KT - 1))
                with nc.allow_low_precision(reason="bf16"):
                    if ncast % 2 == 0:
                        nc.vector.tensor_copy(qk_sb[:, f, ts512], ps[:])
                    else:
                        nc.scalar.activation(qk_sb[:, f, ts512], ps[:], Copy)
                ncast += 1
            # v (natural layout): out [tok 128, feat 512]
            for tt in range(4):
                jt = tq * 4 + tt
                ps = psq.tile([128, 512], f32, tag="v")
                for kt in range(KT):
                    nc.tensor.matmul(
                        ps[:], xt[:, kt, tt * 128:(tt + 1) * 128],
                        wv_sb[:, kt, :],
                        start=(kt == 0), stop=(kt == KT - 1))
                with nc.allow_low_precision(reason="bf16 v"):
                    nc.vector.tensor_copy(
                        v_sb[:, jt, :, 0:D],
                        ps[:].rearrange("p (h d) -> p h d", h=NH))

    # ---- stage 2: attention, per head-pair, query-quarters of 512 ----
    with (
        tc.tile_pool(name="psS", bufs=2, space="PSUM") as psS,
        tc.tile_pool(name="psC", bufs=2, space="PSUM") as psC,
        tc.tile_pool(name="ep", bufs=3) as ep,
        tc.tile_pool(name="dv", bufs=2) as dv,
        tc.tile_pool(name="on", bufs=1) as onp,
    ):
        # ones row at partition 64 for the K=1 denominator-broadcast matmul
        ones_t = onp.tile([65, D], f32r, tag="ones")
        nc.vector.memset(ones_t[:].bitcast(f32), 1.0)
        eidx = 0
        for p in range(4):
            for iq in range(4):
                ihs = slice(iq * 512, (iq + 1) * 512)
                ps_ca = psC.tile([65, 512], f32, tag="CA")
                ps_cb = psC.tile([65, 512], f32, tag="CB")
                # software pipeline: scores(jt) issue before ctx(jt-1)
                pend = None   # (e_a, e_b) waiting for their ctx matmuls
                for jt in range(16):
                    js = slice(jt * 128, (jt + 1) * 128)
                    ps_sa = psS.tile([128, 512], f32, tag="SA")
                    ps_sb = psS.tile([128, 512], f32, tag="SB")
                    for poff, ps_s in ((0, ps_sa), (64, ps_sb)):
                        nc.tensor.matmul(
                            ps_s[:], qk_sb[poff:poff + D, 4 + p, js],
                            qk_sb[poff:poff + D, p, ihs],
                            start=True, stop=True)
                    if pend is not None:
                        for ps_c, e_t, hh in ((ps_ca, pend[0], 0),
                                              (ps_cb, pend[1], 1)):
                            nc.tensor.matmul(
                                ps_c[:], v_sb[:, jt - 1, 2 * p + hh, :],
                                e_t[:], start=(jt == 1), stop=(jt == 16))
                    e_a = ep.tile([128, 512], bf16, tag="EA")
                    e_b = ep.tile([128, 512], bf16, tag="EB")
                    for ps_s, e_t in ((ps_sa, e_a), (ps_sb, e_b)):
                        eng = EXP_SCHED[eidx % len(EXP_SCHED)]
                        eidx += 1
                        with nc.allow_low_precision(reason="exp"):
                            if eng == "s":
                                nc.scalar.activation(e_t[:], ps_s[:], Exp,
                                                     scale=SCALE)
                            else:
                                nc.vector.tensor_scalar(
                                    e_t[:].bitcast(i16), ps_s[:],
                                    A16, B16, Mult, Add)
                    pend = (e_a, e_b)
                # drain last key-chunk's ctx
                for ps_c, e_t, hh in ((ps_ca, pend[0], 0), (ps_cb, pend[1], 1)):
                    nc.tensor.matmul(
                        ps_c[:], v_sb[:, 15, 2 * p + hh, :], e_t[:],
                        start=False, stop=True)
                # softmax division per head (denominator = row 64 of ctx psum)
                for s, ps_c in ((0, ps_ca), (1, ps_cb)):
                    den = dv.tile([65, 512], f32r, tag="den", name=f"den{s}")
                    with nc.allow_low_precision(reason="f32r denom"):
                        nc.vector.tensor_copy(den[64:65, :], ps_c[64:65, :])
                    bc = psS.tile([64, 512], f32, tag="SA", name=f"bc{s}")
                    nc.tensor.matmul(bc[:], ones_t[64:65, :], den[64:65, :],
                                     start=True, stop=True)
                    rec = dv.tile([64, 512], f32, tag="rec", name=f"rec{s}")
                    nc.vector.reciprocal_approx_fast(rec[:], bc[:])
                    with nc.allow_low_precision(reason="bf16 ctx"):
                        if s == 0:
                            nc.vector.tensor_tensor(
                                out=ctx_sb[0:D, p, ihs], in0=ps_c[0:D, :],
                                in1=rec[:], op=Mult)
                        else:
                            tmp = dv.tile([64, 512], bf16, tag="tmp")
                            nc.vector.tensor_tensor(
                                out=tmp[:], in0=ps_c[0:D, :], in1=rec[:],
                                op=Mult)
                            # shift to partitions 64:128 via SBUF->SBUF DMA
                            nc.sync.dma_start(ctx_sb[64:128, p, ihs], tmp[:])

    vp_.release()
    qkp.release()

    # ---- stage 3: output projection (bf16) ----
    with (
        tc.tile_pool(name="w3", bufs=1) as w3,
        tc.tile_pool(name="psO", bufs=2, space="PSUM") as psO,
        tc.tile_pool(name="ot", bufs=3) as otp,
    ):
        wout_sb = w3.tile([128, NPAIR, DM], u16, tag="wout")  # 8 KB/part
        nc.sync.dma_start(wout_sb[:], wo16[:])
        for tt in range(16):
            o_t = otp.tile([128, DM], f32, tag="o")
            for fc in range(2):
                fs = slice(fc * 512, (fc + 1) * 512)
                ps = psO.tile([128, 512], f32, tag="O")
                for pb in range(NPAIR):
                    nc.tensor.matmul(
                        ps[:], ctx_sb[:, pb, tt * 128:(tt + 1) * 128],
                        wout_sb[:, pb, fs].bitcast(bf16),
                        start=(pb == 0), stop=(pb == NPAIR - 1))
                nc.scalar.activation(o_t[:, fs], ps[:], Copy)
            nc.sync.dma_start(out_p[tt * 128:(tt + 1) * 128, :], o_t[:])
    ctp.release()


_CACHE = {}


def _get_nc():
    if "nc" not in _CACHE:
        nc = bacc.Bacc("TRN2", target_bir_lowering=False, debug=False)
        xT = nc.dram_tensor("xT", [128, KT, TOK], f32r, kind="ExternalInput").ap()
        wqkT = nc.dram_tensor("wqkT", [128, KT, 2 * DL], f32r,
                              kind="ExternalInput").ap()
        wvT = nc.dram_tensor("wvT", [128, KT, DL], f32r,
                             kind="ExternalInput").ap()
        wo16 = nc.dram_tensor("wo16", [128, NPAIR, DM], u16,
                              kind="ExternalInput").ap()
        out_p = nc.dram_tensor("out_p", [TOK, DM], f32, kind="ExternalOutput").ap()
        with tile.TileContext(nc) as tc:
            _build(tc, xT, wqkT, wvT, wo16, out_p)
        nc.compile()
        _CACHE["nc"] = nc
    return _CACHE["nc"]


def _bf16_bits(x):
    b = np.ascontiguousarray(np.asarray(x, np.float32)).view(np.uint32)
    return ((b + 0x7FFF + ((b >> 16) & 1)) >> 16).astype(np.uint16)


def _fold(a):
    """[DM, cols] -> [128, KT, cols] with partition-major dm chunks."""
    return np.ascontiguousarray(
        a.reshape(KT, 128, a.shape[1]).transpose(1, 0, 2))


def make_in_maps(x, w_qkv, w_out):
    in_maps = []
    xTb = {b: _fold(np.ascontiguousarray(x[b].T)) for b in range(4)}
    for c in range(N_CORES):
        b, g = c // 2, c % 2
        gs = slice(g * DL, (g + 1) * DL)
        wq = w_qkv[0 * DM + g * DL:0 * DM + (g + 1) * DL]
        wk = w_qkv[1 * DM + g * DL:1 * DM + (g + 1) * DL]
        wv = w_qkv[2 * DM + g * DL:2 * DM + (g + 1) * DL]
        woT = np.ascontiguousarray(w_out[:, gs].T)        # [DL, DM]
        wo16 = np.ascontiguousarray(
            _bf16_bits(woT).reshape(NPAIR, 128, DM).transpose(1, 0, 2))
        in_maps.append({
            "xT": xTb[b],
            "wqkT": _fold(np.ascontiguousarray(np.concatenate([wq, wk], 0).T)),
            "wvT": _fold(np.ascontiguousarray(wv.T)),
            "wo16": wo16,
        })
    return in_maps


def kernel(x, w_qkv, w_out, b_out, _trace=False):
    x = np.asarray(x, dtype=np.float32)
    w_qkv = np.asarray(w_qkv, dtype=np.float32)
    w_out = np.asarray(w_out, dtype=np.float32)
    b_out = np.asarray(b_out, dtype=np.float32)

    nc = _get_nc()
    in_maps = make_in_maps(x, w_qkv, w_out)
    res = bass_utils.run_bass_kernel_spmd(
        nc, in_maps, core_ids=list(range(N_CORES)), trace=_trace)
    out = np.empty((4, TOK, DM), dtype=np.float32)
    for b in range(4):
        out[b] = res.results[2 * b]["out_p"] + res.results[2 * b + 1]["out_p"]
    out += b_out
    if _trace:
        kernel.last_results = res
    return out
